# revision 8
# baseline (speedup 1.0000x reference)
"""
Trainium2 Bass kernel for batched cross-attention:
  context[b] = softmax(q[b] @ tokens[b].T / sqrt(d)) @ tokens[b]
with x_latent (tokens) [16, 4096, 768] f32, prompts_latent (q) [16, 64, 768] f32.

Sharding: data-parallel over batch — 16 batches / 8 cores = 2 per core.

Key ideas vs the previous version (93.7us):
  1. Ship tokens to HBM ONCE (bf16 natural layout [n, d], 12.6 MB/core) instead
     of twice (both layouts, 25.2 MB/core).  The mm1 operand layout [d, n] is
     produced ON-CHIP by PE transposes of the tn tiles (the PE transpose path
     moves ~600 GB/s effective, and the PE has spare cycles).  G1 groups' worth
     of [d, n] tiles are still shipped pre-transposed to balance DMA vs PE.
  2. Col-tiling: the two batches run CONCURRENTLY in disjoint halves of the PE
     array (tile_position (0,0) / (0,64)): batch 0 owns PSUM partitions 0-63,
     batch 1 owns 64-127.  Fixes the M=64 half-array waste of the old kernel.
  3. Softmax row-sums come free from the exp activation's accum_out.

All operands bf16, accumulation f32 => accuracy identical to the old kernel.
"""

import os
import sys

import numpy as np

for _p in ("/opt/trn_rl_repo", "/root/.axon_site/_ro/trn_rl_repo"):
    if os.path.isdir(_p) and _p not in sys.path:
        sys.path.append(_p)

import ml_dtypes
from contextlib import ExitStack

import concourse.bass as bass
import concourse.mybir as mybir
import concourse.tile as tile
from concourse import bacc
from concourse.bass_utils import run_bass_kernel_spmd
from concourse.masks import make_identity

BF16 = ml_dtypes.bfloat16

N_CORES = 8
B_TOTAL = 16
BPC = B_TOTAL // N_CORES  # batches per core
N = 4096  # tokens
D = 768   # latent dim
P = 64    # prompts
DC = D // 128   # d-chunks of 128 (contraction tiles for mm1)
G = N // 512    # groups of 512 token-columns
G1 = 1          # how many (trailing) groups get their [d, n] tiles via DMA
SCALE = float(D) ** -0.5

_cached_nc = None


def build_bass_program() -> bass.Bass:
    nc = bacc.Bacc("TRN2", target_bir_lowering=False, debug=False)
    qt = nc.declare_dram_parameter("qt", [BPC, 128, DC, P], mybir.dt.bfloat16, isOutput=False)
    tn = nc.declare_dram_parameter("tn", [BPC, N, D], mybir.dt.bfloat16, isOutput=False)
    if G1 > 0:
        tt = nc.declare_dram_parameter("tt", [BPC, D, 512 * G1], mybir.dt.bfloat16, isOutput=False)
    out = nc.declare_dram_parameter("out", [BPC, P, D], mybir.dt.float32, isOutput=True)

    with tile.TileContext(nc) as tc, ExitStack() as ctx:
        singles = ctx.enter_context(tc.tile_pool(name="singles", bufs=4))
        tn_pool = ctx.enter_context(tc.tile_pool(name="tn", bufs=8))
        tt_pool = ctx.enter_context(tc.tile_pool(name="tt", bufs=4))
        p_pool = ctx.enter_context(tc.tile_pool(name="pexp", bufs=3))
        pt_pool = ctx.enter_context(tc.tile_pool(name="ptT", bufs=8))
        o_pool = ctx.enter_context(tc.tile_pool(name="osb", bufs=1))
        sc_pool = ctx.enter_context(tc.tile_pool(name="scal", bufs=2))

        psum_s = ctx.enter_context(tc.tile_pool(name="psum_s", bufs=2, space="PSUM"))
        psum_tr = ctx.enter_context(tc.tile_pool(name="psum_tr", bufs=2, space="PSUM"))
        psum_pt = ctx.enter_context(tc.tile_pool(name="psum_pt", bufs=2, space="PSUM"))
        psum_o = ctx.enter_context(tc.tile_pool(name="psum_o", bufs=1, space="PSUM"))

        ident = singles.tile([128, 128], mybir.dt.bfloat16)
        make_identity(nc, ident)

        # queries, pre-swizzled on host to [128, DC, P]
        qt_ts = []
        for b in range(BPC):
            qt_t = singles.tile([128, DC, P], mybir.dt.bfloat16)
            nc.sync.dma_start(out=qt_t, in_=qt[b])
            qt_ts.append(qt_t)

        sums_t = singles.tile([128, G], mybir.dt.float32)

        o_ab = [None]  # lazily allocated PSUM accumulators (o_a [128,512], o_b [128,256])

        tn_r = [
            tn[b].rearrange("(g j p) d -> p g j d", j=4, p=128) for b in range(BPC)
        ]
        if G1 > 0:
            tt_r = [tt[b].rearrange("(c p) n -> p c n", p=128) for b in range(BPC)]

        def load_tn(g):
            tiles = []
            for b in range(BPC):
                t = tn_pool.tile([128, 4, D], mybir.dt.bfloat16, name="tn_g")
                if g == 0:
                    # finer grain so the first transposes start early
                    for j in range(4):
                        nc.sync.dma_start(out=t[:, j], in_=tn_r[b][:, g, j])
                else:
                    nc.sync.dma_start(out=t, in_=tn_r[b][:, g])
                tiles.append(t)
            return tiles

        def make_tt(g, tn_g):
            """Produce tt_g[b] = [128(d-part), DC, 512(n)] bf16 for group g."""
            tts = [
                tt_pool.tile([128, DC, 512], mybir.dt.bfloat16, name="tt_g")
                for _ in range(BPC)
            ]
            if g >= G - G1:
                # shipped pre-transposed from the host
                gg = g - (G - G1)
                for b in range(BPC):
                    if g == 0:
                        for c in range(DC):
                            nc.sync.dma_start(
                                out=tts[b][:, c], in_=tt_r[b][:, c, gg * 512:(gg + 1) * 512]
                            )
                    else:
                        nc.sync.dma_start(
                            out=tts[b], in_=tt_r[b][:, :, gg * 512:(gg + 1) * 512]
                        )
                return tts
            # on-chip transpose: 48 PE transposes of [128, 128] blocks,
            # packed 8-per-PSUM-bank
            k = 0
            tr_tile = None
            for c in range(DC):
                for b in range(BPC):
                    for j in range(4):
                        i = k % 8
                        if i == 0:
                            tr_tile = psum_tr.tile(
                                [128, 8, 128], mybir.dt.bfloat16, name="tr_ps"
                            )
                        nc.tensor.transpose(
                            tr_tile[:, i, :], tn_g[b][:, j, c * 128:(c + 1) * 128], ident
                        )
                        dst = tts[b][:, c, j * 128:(j + 1) * 128]
                        if k % 2 == 0:
                            nc.vector.tensor_copy(dst, tr_tile[:, i, :])
                        else:
                            nc.scalar.copy(dst, tr_tile[:, i, :])
                        k += 1
            return tts

        def mm1(g, tts):
            s_ps = psum_s.tile([128, 512], mybir.dt.float32)
            for c in range(DC):
                for b in range(BPC):
                    nc.tensor.matmul(
                        s_ps[b * P:(b + 1) * P, :],
                        lhsT=qt_ts[b][:, c, :],
                        rhs=tts[b][:, c, :],
                        start=(c == 0),
                        stop=(c == DC - 1),
                        tile_position=(0, b * P),
                    )
            return s_ps

        def softmax_part(g, s_ps):
            p_sb = p_pool.tile([128, 512], mybir.dt.bfloat16)
            acc = sums_t[:, g:g + 1]
            nc.scalar.activation(
                out=p_sb,
                in_=s_ps,
                func=mybir.ActivationFunctionType.Exp,
                scale=SCALE,
                accum_out=acc,
            )
            return p_sb

        def p_transpose(g, p_sb):
            pt_ps = psum_pt.tile([128, 4, 128], mybir.dt.bfloat16, name="pt_ps")
            for j in range(4):
                nc.tensor.transpose(
                    pt_ps[:, j, :], p_sb[:, j * 128:(j + 1) * 128], ident
                )
            outs = []
            for j in range(4):
                pt_sb = pt_pool.tile([128, 128], mybir.dt.bfloat16, name="pt_sb")
                nc.vector.tensor_copy(pt_sb, pt_ps[:, j, :])
                outs.append(pt_sb)
            return outs

        def mm2(g, pts, tn_g):
            if o_ab[0] is None:
                o_a = psum_o.tile([128, 512], mybir.dt.float32, tag="o_a")
                o_b = psum_o.tile([128, 256], mybir.dt.float32, tag="o_b")
                o_ab[0] = (o_a, o_b)
            o_a, o_b = o_ab[0]
            for j in range(4):
                nt = g * 4 + j
                for b in range(BPC):
                    nc.tensor.matmul(
                        o_a[b * P:(b + 1) * P, :],
                        lhsT=pts[j][:, b * P:(b + 1) * P],
                        rhs=tn_g[b][:, j, 0:512],
                        start=(nt == 0),
                        stop=(nt == N // 128 - 1),
                        tile_position=(0, b * P),
                    )
                    nc.tensor.matmul(
                        o_b[b * P:(b + 1) * P, :],
                        lhsT=pts[j][:, b * P:(b + 1) * P],
                        rhs=tn_g[b][:, j, 512:768],
                        start=(nt == 0),
                        stop=(nt == N // 128 - 1),
                        tile_position=(0, b * P),
                    )

        def finish():
            tot = sc_pool.tile([128, 1], mybir.dt.float32)
            nc.vector.reduce_sum(tot, sums_t, axis=mybir.AxisListType.X)
            rec = sc_pool.tile([128, 1], mybir.dt.float32)
            nc.vector.reciprocal(rec, tot)
            o_a, o_b = o_ab[0]
            o_sb = o_pool.tile([128, D], mybir.dt.float32)
            nc.vector.tensor_scalar_mul(o_sb[:, 0:512], o_a, rec)
            nc.vector.tensor_scalar_mul(o_sb[:, 512:768], o_b, rec)
            for b in range(BPC):
                nc.sync.dma_start(out=out[b], in_=o_sb[b * P:(b + 1) * P, :])

        # main software pipeline: PE order per group is
        #   [48 transposes(g)] [12 mm1(g)] [4 P-transposes(g-1)] [16 mm2(g-1)]
        prev = None  # (p_sb, tn_g) of group g-1 awaiting P-transpose + mm2
        tn_tiles = load_tn(0)
        for g in range(G):
            tn_g = tn_tiles
            if g + 1 < G:
                tn_tiles = load_tn(g + 1)
            tts = make_tt(g, tn_g)
            s_ps = mm1(g, tts)
            p_sb = softmax_part(g, s_ps)
            if prev is not None:
                pts = p_transpose(g - 1, prev[0])
                mm2(g - 1, pts, prev[1])
            prev = (p_sb, tn_g)
        pts = p_transpose(G - 1, prev[0])
        mm2(G - 1, pts, prev[1])
        finish()

    nc.compile()
    return nc


def _get_nc() -> bass.Bass:
    global _cached_nc
    if _cached_nc is None:
        _cached_nc = build_bass_program()
    return _cached_nc


def _make_in_maps(x_latent: np.ndarray, prompts_latent: np.ndarray):
    tn_h = np.ascontiguousarray(x_latent.astype(BF16))            # [16, N, D]
    # queries pre-swizzled: [16, 768, 64] -> [16, 128, 6, 64]
    qt_h = prompts_latent.astype(BF16).transpose(0, 2, 1)         # [16, D, P]
    qt_sw = np.ascontiguousarray(
        qt_h.reshape(B_TOTAL, DC, 128, P).transpose(0, 2, 1, 3)
    )  # [16, 128, DC, P]
    maps = []
    if G1 > 0:
        tt_full = tn_h.transpose(0, 2, 1)                         # [16, D, N] view
        tt_part = np.ascontiguousarray(tt_full[:, :, (G - G1) * 512:])
    for c in range(N_CORES):
        m = {
            "qt": qt_sw[c * BPC:(c + 1) * BPC],
            "tn": tn_h[c * BPC:(c + 1) * BPC],
        }
        if G1 > 0:
            m["tt"] = tt_part[c * BPC:(c + 1) * BPC]
        maps.append(m)
    return maps


def run(x_latent: np.ndarray, prompts_latent: np.ndarray, trace: bool = False):
    """Run on all 8 cores; returns (output [16, 64, 768] f32, BassKernelResults)."""
    nc = _get_nc()
    in_maps = _make_in_maps(np.asarray(x_latent), np.asarray(prompts_latent))
    res = run_bass_kernel_spmd(nc, in_maps, list(range(N_CORES)), trace=trace)
    out = np.concatenate([np.asarray(r["out"]) for r in res.results], axis=0)
    return out.astype(np.float32), res


def kernel(x_latent: np.ndarray, prompts_latent: np.ndarray) -> np.ndarray:
    out, _ = run(x_latent, prompts_latent, trace=False)
    return out


# revision 12
# speedup vs baseline: 2.9469x; 2.9469x over previous
"""
Trainium2 Bass kernel for batched cross-attention:
  context[b] = softmax(q[b] @ tokens[b].T / sqrt(d)) @ tokens[b]
with x_latent (tokens) [16, 4096, 768] f32, prompts_latent (q) [16, 64, 768] f32.

Sharding: data-parallel over batch — 16 batches / 8 cores = 2 per core.

Design (v3):
  - Tokens shipped to HBM ONCE in bf16 natural layout (12.6 MB/core), in a
    host pre-tiled layout so every DMA is a single contiguous >=768KB block
    (large descriptors -> near line-rate HBM bandwidth).
  - The mm1 operand layout [d, n] is produced ON-CHIP by PE transposes for
    the first G-G1 groups; the last G1 groups' [d, n] tiles are shipped
    pre-transposed (G1 balances DMA vs PE load).
  - Col-tiling: the two batches run CONCURRENTLY in disjoint halves of the
    PE array (tile_position (0,0)/(0,64)); batch 0 owns PSUM partitions
    0-63, batch 1 owns 64-127.
  - Transposes land in bank-packed PSUM tiles [128, 2, 512]; ONE wide copy
    per (group, c-chunk) moves them to SBUF, rotated across DVE/Pool/ACT.
  - Softmax row-sums come free from the exp activation's accum_out.

All operands bf16, accumulation f32.
"""

import os
import sys

import numpy as np

for _p in ("/opt/trn_rl_repo", "/root/.axon_site/_ro/trn_rl_repo"):
    if os.path.isdir(_p) and _p not in sys.path:
        sys.path.append(_p)

import ml_dtypes
from contextlib import ExitStack

import concourse.bass as bass
import concourse.mybir as mybir
import concourse.tile as tile
from concourse import bacc
from concourse.bass_utils import run_bass_kernel_spmd
from concourse.masks import make_identity

BF16 = ml_dtypes.bfloat16

N_CORES = 8
B_TOTAL = 16
BPC = B_TOTAL // N_CORES  # batches per core
N = 4096  # tokens
D = 768   # latent dim
P = 64    # prompts
DC = D // 128   # d-chunks of 128 (contraction tiles for mm1)
G = N // 512    # groups of 512 token-columns
NPAIR = G // 2  # tn is loaded in pairs of groups (1.5 MB per DMA)
G1 = 2          # trailing groups whose [d, n] tiles come via DMA
NT = N // 128
SCALE = float(D) ** -0.5

_cached_nc = None


def build_bass_program() -> bass.Bass:
    nc = bacc.Bacc("TRN2", target_bir_lowering=False, debug=False)
    qt = nc.declare_dram_parameter("qt", [BPC, 128, DC, P], mybir.dt.bfloat16, isOutput=False)
    tn = nc.declare_dram_parameter("tn", [BPC, NPAIR, 128, 8, D], mybir.dt.bfloat16, isOutput=False)
    if G1 > 0:
        tt = nc.declare_dram_parameter("tt", [BPC, G1, 128, DC, 512], mybir.dt.bfloat16, isOutput=False)
    out = nc.declare_dram_parameter("out", [BPC, P, D], mybir.dt.float32, isOutput=True)

    with tile.TileContext(nc) as tc, ExitStack() as ctx:
        singles = ctx.enter_context(tc.tile_pool(name="singles", bufs=4))
        tn_pool = ctx.enter_context(tc.tile_pool(name="tn", bufs=6))
        tt_pool = ctx.enter_context(tc.tile_pool(name="tt", bufs=2))
        ttd_pool = ctx.enter_context(tc.tile_pool(name="ttd", bufs=2 * max(G1, 1)))
        p_pool = ctx.enter_context(tc.tile_pool(name="pexp", bufs=3))
        pt_pool = ctx.enter_context(tc.tile_pool(name="ptT", bufs=2))
        o_pool = ctx.enter_context(tc.tile_pool(name="osb", bufs=1))
        sc_pool = ctx.enter_context(tc.tile_pool(name="scal", bufs=2))

        psum_s = ctx.enter_context(tc.tile_pool(name="psum_s", bufs=2, space="PSUM"))
        psum_tr = ctx.enter_context(tc.tile_pool(name="psum_tr", bufs=2, space="PSUM"))
        psum_pt = ctx.enter_context(tc.tile_pool(name="psum_pt", bufs=2, space="PSUM"))
        psum_o = ctx.enter_context(tc.tile_pool(name="psum_o", bufs=1, space="PSUM"))

        ident = singles.tile([128, 128], mybir.dt.bfloat16)
        make_identity(nc, ident)

        qt_ts = []
        for b in range(BPC):
            qt_t = singles.tile([128, DC, P], mybir.dt.bfloat16, name="qt_t")
            nc.sync.dma_start(out=qt_t, in_=qt[b])
            qt_ts.append(qt_t)

        sums_t = singles.tile([128, G], mybir.dt.float32, name="sums_t")

        o_ab = [None]

        tn_tiles = {}   # pair -> [tile_b0, tile_b1]
        ttd_tiles = {}  # g -> [tile_b0, tile_b1]

        def load_pair(p, split=False):
            ts = []
            for b in range(BPC):
                t = tn_pool.tile([128, 8, D], mybir.dt.bfloat16, name="tn_p")
                if split:
                    nc.sync.dma_start(out=t[:, 0:4], in_=tn[b, p, :, 0:4])
                    nc.sync.dma_start(out=t[:, 4:8], in_=tn[b, p, :, 4:8])
                else:
                    nc.sync.dma_start(out=t, in_=tn[b, p])
                ts.append(t)
            tn_tiles[p] = ts

        def load_ttd(g):
            gg = g - (G - G1)
            ts = []
            for b in range(BPC):
                t = ttd_pool.tile([128, DC, 512], mybir.dt.bfloat16, name="tt_d")
                nc.sync.dma_start(out=t, in_=tt[b, gg])
                ts.append(t)
            ttd_tiles[g] = ts

        # copy-engine rotation for the big PSUM->SBUF transpose copies
        def big_copy(i, dst, src):
            e = (nc.vector, nc.scalar, nc.vector, nc.vector, nc.scalar, nc.vector)[i % 6]
            if e is nc.scalar:
                e.copy(dst, src)
            else:
                e.tensor_copy(dst, src)

        def make_tt(g):
            """mm1 rhs tiles for group g: [128(d-part), 2(b), 512(n)] per c-chunk,
            packed as one SBUF tile [128, 2, DC, 512]."""
            tn_p = tn_tiles[g // 2]
            jj0 = (g % 2) * 4
            tts = tt_pool.tile([128, 2, DC, 512], mybir.dt.bfloat16, name="tts")
            for c in range(DC):
                tr = psum_tr.tile([128, 2, 512], mybir.dt.bfloat16, name="tr_ps")
                for b in range(BPC):
                    for j in range(4):
                        nc.tensor.transpose(
                            tr[:, b, j * 128:(j + 1) * 128],
                            tn_p[b][:, jj0 + j, c * 128:(c + 1) * 128],
                            ident,
                        )
                big_copy(c, tts[:, :, c, :], tr)
            return tts

        def mm1(g, rhs_of):
            s_ps = psum_s.tile([128, 512], mybir.dt.float32, name="s_ps")
            for c in range(DC):
                for b in range(BPC):
                    nc.tensor.matmul(
                        s_ps[b * P:(b + 1) * P, :],
                        lhsT=qt_ts[b][:, c, :],
                        rhs=rhs_of(b, c),
                        start=(c == 0),
                        stop=(c == DC - 1),
                        tile_position=(0, b * P),
                    )
            return s_ps

        def softmax_part(g, s_ps):
            p_sb = p_pool.tile([128, 512], mybir.dt.bfloat16, name="p_sb")
            nc.scalar.activation(
                out=p_sb,
                in_=s_ps,
                func=mybir.ActivationFunctionType.Exp,
                scale=SCALE,
                accum_out=sums_t[:, g:g + 1],
            )
            return p_sb

        def p_transpose(g, p_sb):
            pt_ps = psum_pt.tile([128, 4, 128], mybir.dt.bfloat16, name="pt_ps")
            for j in range(4):
                nc.tensor.transpose(
                    pt_ps[:, j, :], p_sb[:, j * 128:(j + 1) * 128], ident
                )
            pt_sb = pt_pool.tile([128, 4, 128], mybir.dt.bfloat16, name="pt_sb")
            nc.vector.tensor_copy(pt_sb, pt_ps)
            return pt_sb

        def mm2(g, pt_sb):
            if o_ab[0] is None:
                o_a = psum_o.tile([128, 512], mybir.dt.float32, tag="o_a")
                o_b = psum_o.tile([128, 256], mybir.dt.float32, tag="o_b")
                o_ab[0] = (o_a, o_b)
            o_a, o_b = o_ab[0]
            tn_p = tn_tiles[g // 2]
            jj0 = (g % 2) * 4
            for j in range(4):
                nt = g * 4 + j
                for b in range(BPC):
                    nc.tensor.matmul(
                        o_a[b * P:(b + 1) * P, :],
                        lhsT=pt_sb[:, j, b * P:(b + 1) * P],
                        rhs=tn_p[b][:, jj0 + j, 0:512],
                        start=(nt == 0),
                        stop=(nt == NT - 1),
                        tile_position=(0, b * P),
                    )
                    nc.tensor.matmul(
                        o_b[b * P:(b + 1) * P, :],
                        lhsT=pt_sb[:, j, b * P:(b + 1) * P],
                        rhs=tn_p[b][:, jj0 + j, 512:768],
                        start=(nt == 0),
                        stop=(nt == NT - 1),
                        tile_position=(0, b * P),
                    )

        def finish():
            tot = sc_pool.tile([128, 1], mybir.dt.float32, name="tot")
            nc.vector.reduce_sum(tot, sums_t, axis=mybir.AxisListType.X)
            rec = sc_pool.tile([128, 1], mybir.dt.float32, name="rec")
            nc.vector.reciprocal(rec, tot)
            o_a, o_b = o_ab[0]
            o_sb = o_pool.tile([128, D], mybir.dt.float32, name="o_sb")
            nc.vector.tensor_scalar_mul(o_sb[:, 0:512], o_a, rec)
            nc.vector.tensor_scalar_mul(o_sb[:, 512:768], o_b, rec)
            for b in range(BPC):
                nc.sync.dma_start(out=out[b], in_=o_sb[b * P:(b + 1) * P, :])

        # ---- main pipeline ----
        # DMA schedule: pair0 (split), pair1 upfront; pair2 @g0, pair3 @g2;
        # tt tiles for the G1 trailing groups @g3/g4.
        load_pair(0, split=True)
        load_pair(1)

        prev = None  # (p_sb of g-1)
        for g in range(G):
            if g == 0 and NPAIR > 2:
                load_pair(2)
            if g == 2 and NPAIR > 3:
                load_pair(3)
            if G1 > 0 and 3 <= g < 3 + G1:
                load_ttd(G - G1 + (g - 3))

            if g >= G - G1:
                ts = ttd_tiles[g]
                rhs_of = lambda b, c, ts=ts: ts[b][:, c, :]
            else:
                tts = make_tt(g)
                rhs_of = lambda b, c, tts=tts: tts[:, b, c, :]
            s_ps = mm1(g, rhs_of)
            p_sb = softmax_part(g, s_ps)
            if prev is not None:
                pt_sb = p_transpose(g - 1, prev)
                mm2(g - 1, pt_sb)
            prev = p_sb
        pt_sb = p_transpose(G - 1, prev)
        mm2(G - 1, pt_sb)
        finish()

    nc.compile()
    return nc


def _get_nc() -> bass.Bass:
    global _cached_nc
    if _cached_nc is None:
        _cached_nc = build_bass_program()
    return _cached_nc


def _make_in_maps(x_latent: np.ndarray, prompts_latent: np.ndarray):
    tn_h = np.ascontiguousarray(x_latent.astype(BF16))            # [16, N, D]
    # pre-tiled tn: [16, NPAIR, 128, 8, D]
    tn_sw = np.ascontiguousarray(
        tn_h.reshape(B_TOTAL, NPAIR, 8, 128, D).transpose(0, 1, 3, 2, 4)
    )
    # queries pre-swizzled: [16, 128, DC, P]
    qt_h = prompts_latent.astype(BF16).transpose(0, 2, 1)         # [16, D, P]
    qt_sw = np.ascontiguousarray(
        qt_h.reshape(B_TOTAL, DC, 128, P).transpose(0, 2, 1, 3)
    )
    maps = []
    if G1 > 0:
        tt_full = tn_h.transpose(0, 2, 1)                         # [16, D, N]
        arr = tt_full.reshape(B_TOTAL, DC, 128, G, 512)           # [b, c, p, g, n]
        tt_sw = np.ascontiguousarray(
            arr[:, :, :, G - G1:, :].transpose(0, 3, 2, 1, 4)     # [b, gg, p, c, n]
        )
    for c in range(N_CORES):
        m = {
            "qt": qt_sw[c * BPC:(c + 1) * BPC],
            "tn": tn_sw[c * BPC:(c + 1) * BPC],
        }
        if G1 > 0:
            m["tt"] = tt_sw[c * BPC:(c + 1) * BPC]
        maps.append(m)
    return maps


def run(x_latent: np.ndarray, prompts_latent: np.ndarray, trace: bool = False):
    """Run on all 8 cores; returns (output [16, 64, 768] f32, BassKernelResults)."""
    nc = _get_nc()
    in_maps = _make_in_maps(np.asarray(x_latent), np.asarray(prompts_latent))
    res = run_bass_kernel_spmd(nc, in_maps, list(range(N_CORES)), trace=trace)
    out = np.concatenate([np.asarray(r["out"]) for r in res.results], axis=0)
    return out.astype(np.float32), res


def kernel(x_latent: np.ndarray, prompts_latent: np.ndarray) -> np.ndarray:
    out, _ = run(x_latent, prompts_latent, trace=False)
    return out
